# revision 1
# baseline (speedup 1.0000x reference)
"""Causal self-attention (GPT-style block) on 8 Trainium2 NeuronCores.

Sharding: tensor-parallel over heads. 16 heads / 8 cores = 2 heads per core.
- c_attn column-parallel: each core computes q/k/v for its 2 heads (128
  channels each of q, k, v) from the full input x.
- attention: fully local per core (its 2 heads, all 4 batches).
- c_proj token-parallel after an on-device AllToAll of the attention
  output (see below); each core returns fully-reduced output rows for its
  own token shard, and the host just concatenates and adds b_proj.

Device kernel notes (all matmuls contract over the partition dim):
- Matmul inputs use float32r (single-pass fp32 on the PE, 4x the fp32 rate;
  ~1.5e-4 input rounding, fp32 accumulate).
- x is fed pre-transposed + tiled from the host: xp[tb, p, kt, s] =
  x[(tb*512+s) token, (kt*128+p) channel] so stage 1 needs no transposes.
- q,k,v are produced channel-major ([chan, token]); v is then PE-transposed
  to token-major tiles with a ones column appended (vaug[.., 64]==1), so a
  single M=65 matmul accumulates both O^T = V^T E and the softmax
  denominator (row 64) per key tile.
- Scores are computed transposed: S^T[key, query] = (k^T).T @ q^T with the
  2 heads packed into the two 64-row halves of the PE array (row tiling).
- Softmax without max-subtraction (logits bounded ~|3| here): E =
  exp(S^T/8) on ACT, causal mask applied multiplicatively on the 4 partial
  (diagonal) key-tiles per query block.
- Normalization: r = 1/l on DVE, broadcast across the 64 head rows with a
  K=1 ones matmul on PE, multiply on DVE. Result lands channel-major in
  yT, which is exactly the stationary layout c_proj needs.
- c_proj is token-parallel: per half-batch (1024 tokens), an on-device
  AllToAll exchanges Y^T slices (each core sends peer j its 2 head-channels
  for peer j's 128 tokens of that half), after which every core holds all
  1024 channels for its own tokens and computes fully-reduced output rows
  with the full w_proj. This cuts per-core PSUM->SBUF eviction and output
  DMA 8x vs row-parallel partial sums. Exchanges are half-batch sized so
  they start mid-batch, and each unit's projection is emitted 3 exchange
  units later so collective latency hides under subsequent attention work
  (engine instruction streams execute in order, so a not-yet-ready
  projection would otherwise stall everything emitted after it).
- The LAST half-batch skips the exchange: waiting out the final
  collective would leave the PE idle with nothing left to overlap, so that
  unit's projection is computed row-parallel (each core: its 128 channels
  x its w_proj row slice -> a partial over all 1024 of its tokens) right
  after the final normalize while the PE is still warm, and the host sums
  those 8 partials for just that token range.
"""

import numpy as np

P = 128
B = 4
T = 2048
BT = B * T            # 8192 tokens
C = 1024
KT = C // P           # 8 contraction tiles of 128 input channels
NTB = BT // 512       # 16 token blocks of 512
HD = 64               # head dim
NQ = T // 512         # 4 query blocks per batch
NCORES = 8

_CACHED = {}


def _build_nc():
    import concourse.mybir as mybir
    import concourse.tile as tile
    from concourse import bacc
    from concourse.masks import make_identity

    f32 = mybir.dt.float32
    f32r = mybir.dt.float32r
    EXP = mybir.ActivationFunctionType.Exp

    nc = bacc.Bacc("TRN2", target_bir_lowering=False, debug=False,
                   num_devices=NCORES)

    TPC = T // NCORES   # 256 tokens per core per batch (proj sharding)

    xp = nc.dram_tensor("xp", [NTB, P, KT, 512], f32r, kind="ExternalInput")
    wq = nc.dram_tensor("wq", [P, KT, P], f32r, kind="ExternalInput")
    wk = nc.dram_tensor("wk", [P, KT, P], f32r, kind="ExternalInput")
    wv = nc.dram_tensor("wv", [P, KT, P], f32r, kind="ExternalInput")
    wp = nc.dram_tensor("wp", [P, KT, C], f32r, kind="ExternalInput")
    bq = nc.dram_tensor("bq", [P, 1], f32, kind="ExternalInput")
    bk = nc.dram_tensor("bk", [P, 1], f32, kind="ExternalInput")
    bv = nc.dram_tensor("bv", [P, 1], f32, kind="ExternalInput")
    wpr = nc.dram_tensor("wpr", [P, C], f32r, kind="ExternalInput")
    yp = nc.dram_tensor("yp", [B, 2, T // 2 // NCORES, C], f32, kind="ExternalOutput")
    ypl = nc.dram_tensor("ypl", [T // 2, C], f32, kind="ExternalOutput")

    with tile.TileContext(nc) as tc:
        with (
            tc.tile_pool(name="const", bufs=1) as const,
            tc.tile_pool(name="xt", bufs=2) as xt_pool,
            tc.tile_pool(name="slab", bufs=2) as slab_pool,
            tc.tile_pool(name="e", bufs=5) as e_pool,
            tc.tile_pool(name="nrm", bufs=2) as nrm_pool,
            tc.tile_pool(name="ob", bufs=3) as ob_pool,
            tc.tile_pool(name="yg", bufs=3) as yg_pool,
            tc.tile_pool(name="dram", bufs=1, space="DRAM") as dram_pool,
            tc.tile_pool(name="ps1", bufs=1, space="PSUM") as ps1_pool,
            tc.tile_pool(name="pss", bufs=2, space="PSUM") as pss_pool,
            tc.tile_pool(name="pso", bufs=2, space="PSUM") as pso_pool,
            tc.tile_pool(name="ppb", bufs=1, space="PSUM") as ppb_pool,
        ):
            TPH = TPC // 2   # 128 tokens per core per half-batch exchange
            g_in = [dram_pool.tile([NCORES, P, TPH], f32r, name=f"g_in{k}",
                                   tag=f"g_in{k}") for k in range(2 * B)]
            g_out = [dram_pool.tile([NCORES, P, TPH], f32r, name=f"g_out{k}",
                                    tag=f"g_out{k}") for k in range(2 * B)]

            # --- constants / weights resident in SBUF ---
            wq_sb = const.tile([P, KT, P], f32r)
            wk_sb = const.tile([P, KT, P], f32r)
            wv_sb = const.tile([P, KT, P], f32r)
            wp_sb = const.tile([P, KT, C], f32r)
            wpr_sb = const.tile([P, C], f32r)
            bq_sb = const.tile([P, 1], f32)
            bk_sb = const.tile([P, 1], f32)
            bv_sb = const.tile([P, 1], f32)
            nc.sync.dma_start(wq_sb[:], wq[:])
            nc.sync.dma_start(bq_sb[:], bq[:])
            nc.sync.dma_start(bk_sb[:], bk[:])
            nc.sync.dma_start(bv_sb[:], bv[:])

            ones_row_f = const.tile([1, HD], f32)
            nc.vector.memset(ones_row_f[:], 1.0)
            ones_row = const.tile([1, HD], f32r)
            nc.vector.tensor_copy(ones_row[:], ones_row_f[:])
            ones_v = const.tile([P, T // P, 1], f32)
            nc.vector.memset(ones_v[:], 1.0)
            ident = const.tile([P, P], f32)
            make_identity(nc, ident[:])

            # mask[p, s] = 1.0 if s >= p else 0.0 (keep upper-right triangle)
            # (built in f32 — gpsimd can't write f32r — then rounded over)
            mask_f = const.tile([P, P], f32)
            nc.gpsimd.memset(mask_f[:], 1.0)
            nc.gpsimd.affine_select(
                out=mask_f[:],
                in_=mask_f[:],
                compare_op=mybir.AluOpType.is_ge,
                fill=0.0,
                base=0,
                pattern=[[1, P]],
                channel_multiplier=-1,
            )
            mask_sb = const.tile([P, P], f32r)
            nc.vector.tensor_copy(mask_sb[:], mask_f[:])

            wp_loaded = []

            def emit_proj(k):
                if not wp_loaded:
                    # deferred so the 4MiB w_proj load doesn't delay the
                    # startup xp streaming
                    nc.sync.dma_start(wp_sb[:], wp[:])
                    nc.sync.dma_start(wpr_sb[:], wpr[:])
                    wp_loaded.append(True)
                # yg[p, cc, t]: channel cc*128+p of my token t of unit k
                yg = yg_pool.tile([P, NCORES, TPH], f32r, tag="yg")
                nc.sync.dma_start(yg[:], g_out[k].rearrange("c p t -> p c t"))
                pp0 = ppb_pool.tile([P, 512], f32, tag="ppb")
                pp1 = ppb_pool.tile([P, 512], f32, tag="ppb")
                for ct in range(KT):
                    nc.tensor.matmul(pp0[:], yg[:, ct, :], wp_sb[:, ct, 0:512],
                                     start=(ct == 0), stop=(ct == KT - 1))
                for ct in range(KT):
                    nc.tensor.matmul(pp1[:], yg[:, ct, :], wp_sb[:, ct, 512:C],
                                     start=(ct == 0), stop=(ct == KT - 1))
                ob = ob_pool.tile([P, C], f32, tag="ob")
                nc.vector.tensor_copy(ob[:, 0:512], pp0[:])
                nc.scalar.copy(ob[:, 512:C], pp1[:])
                nc.sync.dma_start(yp[k // 2, k % 2, :, :], ob[:])

            def emit_proj_partial(yTh):
                # ping-pong between two psum pools so the tail chain is
                # paced by DVE evictions, not psum-slot round trips
                for tt in range(T // 2 // P):
                    tsl = slice(tt * P, (tt + 1) * P)
                    pp0 = ppb_pool.tile([P, 512], f32, tag="ppb")
                    pp1 = ps1_pool.tile([P, 512], f32, tag="ps1")
                    nc.tensor.matmul(pp0[:], yTh[:, tsl], wpr_sb[:, 0:512],
                                     start=True, stop=True)
                    nc.tensor.matmul(pp1[:], yTh[:, tsl], wpr_sb[:, 512:C],
                                     start=True, stop=True)
                    ob = ob_pool.tile([P, C], f32, tag="ob")
                    nc.vector.tensor_copy(ob[:, 0:512], pp0[:])
                    nc.scalar.copy(ob[:, 512:C], pp1[:])
                    nc.sync.dma_start(ypl[tsl, :], ob[:])

            def emit_exchange(k, yTh):
                # peer j gets my 2 head-channels for its 128 tokens of unit k
                for j in range(NCORES):
                    nc.sync.dma_start(g_in[k][j], yTh[:, j * TPH:(j + 1) * TPH])
                nc.gpsimd.collective_compute(
                    "AllToAll",
                    mybir.AluOpType.bypass,
                    replica_groups=[list(range(NCORES))],
                    ins=[g_in[k][:]],
                    outs=[g_out[k][:]],
                )

            for b in range(B):
                # --- stage 1: q^T, k^T, v^T (channel-major, f32r) ---
                qT = slab_pool.tile([P, T], f32r, tag="qT")
                kT = slab_pool.tile([P, T], f32r, tag="kT")
                vT = slab_pool.tile([P, T], f32, tag="scratch")
                # token-major v with ones cols at 64 (h0) and 129 (h1)
                vaug = slab_pool.tile([P, T // P, 2 * HD + 2], f32r, tag="vaug")
                nc.vector.tensor_copy(vaug[:, :, HD:HD + 1], ones_v[:])
                nc.vector.tensor_copy(vaug[:, :, 2 * HD + 1:2 * HD + 2], ones_v[:])

                for lb in range(NQ):
                    tb = b * NQ + lb
                    xt = xt_pool.tile([P, KT, 512], f32r)
                    if tb == 0:
                        # first block on the gpsimd queue, parallel to the
                        # weight loads on the sync queue
                        nc.gpsimd.dma_start(xt[:], xp[tb])
                    else:
                        nc.sync.dma_start(xt[:], xp[tb])
                    if tb == 0:
                        # behind the first x block: k/v weights aren't needed
                        # until after the first q matmul group
                        nc.sync.dma_start(wk_sb[:], wk[:])
                        nc.sync.dma_start(wv_sb[:], wv[:])
                    sl = slice(lb * 512, (lb + 1) * 512)

                    for w_sb, b_sb, dst in ((wq_sb, bq_sb, qT),
                                            (wk_sb, bk_sb, kT),
                                            (wv_sb, bv_sb, vT)):
                        ps = ps1_pool.tile([P, 512], f32, tag="ps1")
                        for kt in range(KT):
                            nc.tensor.matmul(ps[:], w_sb[:, kt, :], xt[:, kt, :],
                                             start=(kt == 0), stop=(kt == KT - 1))
                        nc.vector.tensor_scalar_add(dst[:, sl], ps[:], b_sb[:])

                    # transpose v to token-major [tok, chan] tiles
                    for t4 in range(4):
                        j = lb * 4 + t4
                        pst = ps1_pool.tile([P, P], f32, tag="ps1")
                        nc.tensor.transpose(pst[:], vT[:, j * P:(j + 1) * P], ident[:])
                        nc.vector.tensor_copy(vaug[:, j, 0:HD], pst[:, 0:HD])
                        nc.vector.tensor_copy(vaug[:, j, HD + 1:2 * HD + 1],
                                              pst[:, HD:P])

                # --- stage 2: attention, per query block ---
                for i in range(NQ):
                    if i % 2 == 0:
                        yT = slab_pool.tile([P, T // 2], f32r, tag="scratch",
                                            name=f"yT_{b}_{i // 2}")
                    isl = slice((i % 2) * 512, (i % 2 + 1) * 512)
                    nj = 4 * (i + 1)
                    po0 = pso_pool.tile([P, 512], f32, tag="pso")
                    po1 = pso_pool.tile([P, 512], f32, tag="pso")

                    def ranges(j):
                        # diagonal tiles: queries below q0 can't see this key
                        # tile — compute only the [q0, 512) query range
                        q0 = max(0, j - 4 * i) * P
                        return q0, slice(q0, 512), slice(512 + q0, 1024)

                    def emit_s(j):
                        # both heads' scores side by side in one 2-bank
                        # psum tile -> a single exp per key tile
                        q0, vsl, v1 = ranges(j)
                        jsl = slice(j * P, (j + 1) * P)
                        qsl = slice(i * 512 + q0, (i + 1) * 512)
                        psp = pss_pool.tile([P, 1024], f32, tag="pss",
                                            name=f"psp{j % 2}")
                        nc.tensor.matmul(psp[:, vsl], kT[0:HD, jsl], qT[0:HD, qsl],
                                         start=True, stop=True, tile_position=(0, 0))
                        nc.tensor.matmul(psp[:, v1], kT[HD:P, jsl], qT[HD:P, qsl],
                                         start=True, stop=True, tile_position=(HD, 0))
                        ep = e_pool.tile([P, 1024], f32r, tag="e", name=f"ep{j % 2}")
                        if q0 == 0:
                            nc.scalar.activation(ep[:], psp[:], EXP, scale=0.125)
                        else:
                            nc.scalar.activation(ep[:, vsl], psp[:, vsl], EXP,
                                                 scale=0.125)
                            nc.scalar.activation(ep[:, v1], psp[:, v1], EXP,
                                                 scale=0.125)
                        if j - 4 * i >= 0:
                            for q in (q0, 512 + q0):
                                msl = slice(q, q + P)
                                nc.vector.tensor_mul(ep[:, msl], ep[:, msl],
                                                     mask_sb[:])
                        return ep

                    # software-pipelined: S(j+1) is issued before PV(j) so the
                    # PE never sits directly behind ACT's exp latency
                    eps = {0: emit_s(0)}
                    for j in range(nj):
                        if j + 1 < nj:
                            eps[j + 1] = emit_s(j + 1)
                        ep = eps.pop(j)
                        q0, vsl, v1 = ranges(j)
                        st, sp = (j == 0), (j == nj - 1)
                        nc.tensor.matmul(po0[0:HD + 1, vsl], vaug[:, j, 0:HD + 1],
                                         ep[:, vsl], start=st, stop=sp)
                        nc.tensor.matmul(po1[0:HD + 1, vsl],
                                         vaug[:, j, HD + 1:2 * HD + 2], ep[:, v1],
                                         start=st, stop=sp)

                    # normalize: yT[head, isl] = O^T * (1/l) broadcast over rows
                    with nc.allow_low_precision(reason="f32r matmul inputs"):
                        r0 = nrm_pool.tile([1, 512], f32r, tag="r")
                        r1 = nrm_pool.tile([1, 512], f32r, tag="r")
                        nc.vector.reciprocal(r0[:], po0[HD:HD + 1, :])
                        nc.vector.reciprocal(r1[:], po1[HD:HD + 1, :])
                    pb0 = ppb_pool.tile([P, 512], f32, tag="ppb")
                    pb1 = ppb_pool.tile([P, 512], f32, tag="ppb")
                    nc.tensor.matmul(pb0[0:HD, :], ones_row[:], r0[:],
                                     start=True, stop=True)
                    nc.tensor.matmul(pb1[0:HD, :], ones_row[:], r1[:],
                                     start=True, stop=True)
                    rb0 = nrm_pool.tile([HD, 512], f32, tag="rb")
                    rb1 = nrm_pool.tile([HD, 512], f32, tag="rb")
                    nc.vector.tensor_copy(rb0[:], pb0[0:HD, :])
                    nc.vector.tensor_copy(rb1[:], pb1[0:HD, :])
                    nc.vector.tensor_mul(yT[0:HD, isl], po0[0:HD, :], rb0[:])
                    nc.vector.tensor_mul(yT[HD:P, isl], po1[0:HD, :], rb1[:])

                    if i % 2 == 1:
                        k = 2 * b + i // 2
                        if k < 2 * B - 1:
                            emit_exchange(k, yT)
                            if k >= 3:
                                emit_proj(k - 3)
                        else:
                            # tail: the three ready exchange-projections go
                            # first (PE-dense), the DVE-paced partial drains
                            # behind them
                            for kk in range(2 * B - 4, 2 * B - 1):
                                emit_proj(kk)
                            emit_proj_partial(yT)

    nc.compile()
    return nc


def _prep_inputs(x, w_attn, b_attn, w_proj):
    x = np.asarray(x, dtype=np.float32)
    w_attn = np.asarray(w_attn, dtype=np.float32)
    b_attn = np.asarray(b_attn, dtype=np.float32)
    w_proj = np.asarray(w_proj, dtype=np.float32)

    x_flat = x.reshape(BT, C)
    # xp[tb, p, kt, s] = x_flat[tb*512+s, kt*128+p]
    xp = np.ascontiguousarray(
        x_flat.T.reshape(KT, P, NTB, 512).transpose(2, 1, 0, 3))

    wp = np.ascontiguousarray(w_proj.reshape(KT, P, C).transpose(1, 0, 2))
    in_maps = []
    for c in range(NCORES):
        cols = slice(P * c, P * (c + 1))

        def wslice(off):
            w = w_attn[:, off + P * c: off + P * (c + 1)]   # [1024, 128]
            return np.ascontiguousarray(w.reshape(KT, P, P).transpose(1, 0, 2))

        in_maps.append({
            "xp": xp,
            "wq": wslice(0),
            "wk": wslice(C),
            "wv": wslice(2 * C),
            "wp": wp,
            "wpr": np.ascontiguousarray(w_proj[cols, :]),
            "bq": np.ascontiguousarray(b_attn[cols]).reshape(P, 1),
            "bk": np.ascontiguousarray(b_attn[C + P * c: C + P * (c + 1)]).reshape(P, 1),
            "bv": np.ascontiguousarray(b_attn[2 * C + P * c: 2 * C + P * (c + 1)]).reshape(P, 1),
        })
    return in_maps


def kernel(x, w_attn, b_attn, w_proj, b_proj):
    from concourse.bass_utils import run_bass_kernel_spmd

    if "nc" not in _CACHED:
        _CACHED["nc"] = _build_nc()
    nc = _CACHED["nc"]

    in_maps = _prep_inputs(x, w_attn, b_attn, w_proj)
    res = run_bass_kernel_spmd(nc, in_maps, core_ids=list(range(NCORES)))

    # core c holds tokens [h*1024 + c*128, +128) of each batch half h,
    # except the last half-batch which comes back as row-parallel partials
    y = np.empty((B, T, C), dtype=np.float32)
    for c in range(NCORES):
        part = res.results[c]["yp"]          # [B, 2, 128, C]
        for h in range(2):
            y[:, h * (T // 2) + c * 128: h * (T // 2) + (c + 1) * 128, :] = part[:, h]
    acc = res.results[0]["ypl"].astype(np.float32).copy()
    for c in range(1, NCORES):
        acc += res.results[c]["ypl"]
    y[B - 1, T // 2:, :] = acc
    y += np.asarray(b_proj, dtype=np.float32)
    return y



# revision 3
# speedup vs baseline: 1.2212x; 1.2212x over previous
"""Causal self-attention (GPT-style block) on 8 Trainium2 NeuronCores.

Sharding: tensor-parallel over heads (16 heads / 8 cores = 2 per core),
c_attn column-parallel from the full input x, attention fully local per
core, c_proj token-parallel after an on-device AllToAll (all structure
inherited from the f32r baseline).

What changed vs the f32r baseline (325.9us) — mixed-precision rebalance
driven by the TimelineSim cost model (matmul cost = out_free_rows x
cycles_per_row; fp8 DoubleRow = 0.5 c/r; ACT exp = 0.833 ns/row and is
the true floor at ~17.4M exps/core):

- Value path (v, E*V, y, w_proj) in bf16: fp8 there costs 2.6-3.7e-2
  rel err (gate 2e-2); score path tolerates fp8 thanks to the 1/8
  softmax scale (measured 1.4e-2 end to end for this config).
- q,k generation: fp8 DoubleRow (contraction 2x128 per pass), from an
  fp8 copy of x^T and w_qk pre-scaled by 16 on the host (fp8 subnormal
  floor); the 1/256 compensation is folded into the exp scale.
- v generation: bf16, emitted TOKEN-major directly (x^T tile as the
  stationary) so the PE transposes + vaug copies of the baseline die.
- Scores: fp8 DoubleRow with the head-dim-64 contraction zero-padded in
  the second k-subtile (cost model charges out_rows*0.5 regardless).
  S psum tiles are [128, 1024] (tile j x 2 heads); diagonal tiles clip
  queries below the 128-granular diagonal, and exp ranges merge across
  the d=1 gap (garbage cols are never consumed).
- PV: fat-M orientation — out po[q-tile 128, 65] per (key tile, head),
  ap=65/key-tile instead of 512 (2.2x fewer PE rows than the baseline
  PV), accumulated across key tiles in a 2-bank psum with a single
  start per bank (zero-region covers the other slots). Column 65 of
  the bf16 token-major V carries ones so the same matmul accumulates
  the softmax denominators.
- Normalize: gpsimd InstNormalizeRecip (out[i,j]=in[i,j]/denom[i], bf16
  cast at write) — kills the baseline's PE broadcast + DVE chain.
- y^T for c_proj via 4 PE transposes per block (bf16 identity).
- exp is the ONLY thing on ACT (one table load, no evictions there).
- Exchange + c_proj: baseline machinery in bf16 (half the collective
  bytes); last half-batch row-parallel partials emitted bf16, summed on
  the host. b_v is folded into a host-side output bias (softmax weights
  sum to 1 => v-bias shifts y by b_v exactly); b_q/b_k stay on-device.
- Stage-1 of batch b+1 is interleaved into attention of batch b at
  block granularity so ACT never drains between batches.
"""

import numpy as np
import ml_dtypes

P = 128
B = 4
T = 2048
BT = B * T            # 8192 tokens
C = 1024
KT = C // P           # 8 contraction tiles of 128
KT2 = KT // 2         # 4 DoubleRow pairs
NTB = BT // 512       # 16 token blocks of 512
HD = 64               # head dim
NQ = T // 512         # 4 query blocks per batch
NCORES = 8
TPC = T // NCORES     # 256 tokens per core per batch (proj sharding)
TPH = TPC // 2        # 128 tokens per core per half-batch exchange
WS = 16.0             # host prescale on w_q/w_k (fp8 subnormal floor)
SEXP = 0.125 / (WS * WS)

E4NP = ml_dtypes.float8_e4m3
BFNP = ml_dtypes.bfloat16

_CACHED = {}


def _exp_ranges(q0):
    # valid score cols per [tile j | 2 heads] psum tile, merged where the
    # gap garbage (never consumed by PV) is cheaper than an extra ACT
    # instruction (~185ns ~ 222 rows)
    if q0 == 0:
        return [(0, 1024)]
    if q0 == 128:
        return [(q0, 1024)]
    return [(q0, 512), (512 + q0, 1024)]


def _build_nc():
    import concourse.mybir as mybir
    import concourse.tile as tile
    from concourse import bacc
    from concourse.masks import make_identity

    f32 = mybir.dt.float32
    bf16 = mybir.dt.bfloat16
    f8 = mybir.dt.float8e4
    EXP = mybir.ActivationFunctionType.Exp
    DR = mybir.MatmulPerfMode.DoubleRow

    nc = bacc.Bacc("TRN2", target_bir_lowering=False, debug=False,
                   num_devices=NCORES)

    xp8 = nc.dram_tensor("xp8", [NTB, P, KT2, 2, 512], f8, kind="ExternalInput")
    xpb = nc.dram_tensor("xpb", [NTB, P, KT, 512], bf16, kind="ExternalInput")
    wq8 = nc.dram_tensor("wq8", [P, KT2, 2, P], f8, kind="ExternalInput")
    wk8 = nc.dram_tensor("wk8", [P, KT2, 2, P], f8, kind="ExternalInput")
    wvb = nc.dram_tensor("wvb", [P, KT, P], bf16, kind="ExternalInput")
    wpb = nc.dram_tensor("wpb", [P, KT, C], bf16, kind="ExternalInput")
    wprb = nc.dram_tensor("wprb", [P, C], bf16, kind="ExternalInput")
    bq = nc.dram_tensor("bq", [P, 1], f32, kind="ExternalInput")
    bk = nc.dram_tensor("bk", [P, 1], f32, kind="ExternalInput")
    yp = nc.dram_tensor("yp", [B, 2, TPH, C], f32, kind="ExternalOutput")
    ypl = nc.dram_tensor("ypl", [T // 2, C], bf16, kind="ExternalOutput")

    with tile.TileContext(nc) as tc:
        with (
            tc.tile_pool(name="const", bufs=1) as const,
            tc.tile_pool(name="slab", bufs=1) as slab,
            tc.tile_pool(name="yt", bufs=2) as yt_pool,
            tc.tile_pool(name="x8", bufs=2) as x8_pool,
            tc.tile_pool(name="xb", bufs=2) as xb_pool,
            tc.tile_pool(name="e", bufs=5) as e_pool,
            tc.tile_pool(name="pb", bufs=2) as posb_pool,
            tc.tile_pool(name="y8", bufs=2) as y8b_pool,
            tc.tile_pool(name="yg", bufs=3) as yg_pool,
            tc.tile_pool(name="ob", bufs=3) as ob_pool,
            tc.tile_pool(name="dram", bufs=1, space="DRAM") as dram_pool,
            tc.tile_pool(name="pss", bufs=2, space="PSUM") as pss_pool,
            tc.tile_pool(name="shp", bufs=2, space="PSUM") as shp_pool,
            tc.tile_pool(name="pop", bufs=1, space="PSUM") as pop_pool,
        ):
            g_in = [dram_pool.tile([NCORES, P, TPH], bf16, name=f"g_in{k}",
                                   tag=f"g_in{k}") for k in range(2 * B)]
            g_out = [dram_pool.tile([NCORES, P, TPH], bf16, name=f"g_out{k}",
                                    tag=f"g_out{k}") for k in range(2 * B)]

            # --- constants / weights resident in SBUF ---
            wq8_sb = const.tile([P, KT2, 2, P], f8)
            wk8_sb = const.tile([P, KT2, 2, P], f8)
            wvb_sb = const.tile([P, KT, P], bf16)
            wpb_sb = const.tile([P, KT, C], bf16)
            wprb_sb = const.tile([P, C], bf16)
            bq_sb = const.tile([P, 1], f32)
            bk_sb = const.tile([P, 1], f32)
            nc.sync.dma_start(wq8_sb[:], wq8[:])
            nc.sync.dma_start(wk8_sb[:], wk8[:])
            nc.sync.dma_start(bq_sb[:], bq[:])
            nc.sync.dma_start(bk_sb[:], bk[:])
            nc.sync.dma_start(wvb_sb[:], wvb[:])

            identf = const.tile([P, P], f32)
            make_identity(nc, identf[:])
            identb = const.tile([P, P], bf16)
            nc.vector.tensor_copy(identb[:], identf[:])

            # mask[p, u] = 1.0 if u >= p else 0.0 (upper-right triangle)
            mask_f = const.tile([P, P], f32)
            nc.gpsimd.memset(mask_f[:], 1.0)
            nc.gpsimd.affine_select(
                out=mask_f[:],
                in_=mask_f[:],
                compare_op=mybir.AluOpType.is_ge,
                fill=0.0,
                base=0,
                pattern=[[1, P]],
                channel_multiplier=-1,
            )
            maskb = const.tile([P, P], bf16)
            nc.vector.tensor_copy(maskb[:], mask_f[:])

            # double-buffered (even/odd batch) stage-1 slabs
            qT8 = [slab.tile([P, 2, T], f8, name=f"qT8_{e}", tag=f"qT8_{e}")
                   for e in range(2)]
            kT8 = [slab.tile([P, 2, T], f8, name=f"kT8_{e}", tag=f"kT8_{e}")
                   for e in range(2)]
            vaug = [slab.tile([P, NQ * 4, 2, HD + 1], bf16, name=f"vaug_{e}",
                              tag=f"vaug_{e}") for e in range(2)]
            for e in range(2):
                # zero second k-subtile (DoubleRow pads head-dim 64 -> 2x64)
                nc.gpsimd.memset(qT8[e][:, 1, :], 0.0)
                nc.gpsimd.memset(kT8[e][:, 1, :], 0.0)
                # ones column drives the softmax denominators through PV
                nc.gpsimd.memset(vaug[e][:, :, :, HD:HD + 1], 1.0)

            wp_loaded = []

            def emit_stage1(g):
                b, lb = g // 4, g % 4
                xt8 = x8_pool.tile([P, KT2, 2, 512], f8, name=f"xt8_{g}",
                                   tag="xt8")
                xtb = xb_pool.tile([P, KT, 512], bf16, name=f"xtb_{g}",
                                   tag="xtb")
                if g == 0:
                    nc.gpsimd.dma_start(xt8[:], xp8[g])
                    nc.gpsimd.dma_start(xtb[:], xpb[g])
                else:
                    nc.sync.dma_start(xt8[:], xp8[g])
                    nc.sync.dma_start(xtb[:], xpb[g])
                sl = slice(lb * 512, (lb + 1) * 512)
                # q, k: fp8 DoubleRow, channel-major out
                for w_sb, b_sb, dst in ((wq8_sb, bq_sb, qT8[b % 2]),
                                        (wk8_sb, bk_sb, kT8[b % 2])):
                    ps = shp_pool.tile([P, 512], f32, tag="shp",
                                       name=f"ps_{g}")
                    for k2 in range(KT2):
                        nc.tensor.matmul(ps[:], w_sb[:, k2], xt8[:, k2],
                                         start=(k2 == 0), stop=(k2 == KT2 - 1),
                                         perf_mode=DR)
                    nc.vector.tensor_scalar_add(dst[:, 0, sl], ps[:], b_sb[:])
                # v: bf16, token-major (x^T tile stationary); single psum
                # bank, one start — zero-region covers the other tt slots
                vps = shp_pool.tile([P, 4, 2, HD], f32, tag="shp",
                                    name=f"vps_{g}")
                for tt in range(4):
                    for kt in range(KT):
                        nc.tensor.matmul(vps[:, tt], xtb[:, kt, tt * P:(tt + 1) * P],
                                         wvb_sb[:, kt, :],
                                         start=(tt == 0 and kt == 0),
                                         stop=(tt == 3 and kt == KT - 1))
                nc.vector.tensor_copy(vaug[b % 2][:, lb * 4:(lb + 1) * 4, :, 0:HD],
                                      vps[:])

            def emit_s(b, i, j):
                # S^T[key, query] for both heads of tile j, fp8 DoubleRow
                d = j - 4 * i
                q0 = max(0, d) * P
                qb_, kb_ = qT8[b % 2], kT8[b % 2]
                psp = pss_pool.tile([P, 1024], f32, tag="pss",
                                    name=f"psp_{b}_{i}_{j}")
                for h in range(2):
                    nc.tensor.matmul(
                        psp[:, 512 * h + q0:512 * h + 512],
                        kb_[HD * h:HD * h + HD, :, j * P:(j + 1) * P],
                        qb_[HD * h:HD * h + HD, :, i * 512 + q0:(i + 1) * 512],
                        start=True, stop=True, perf_mode=DR,
                        tile_position=(HD * h, 0))
                ep = e_pool.tile([P, 1024], bf16, tag="e",
                                 name=f"ep_{b}_{i}_{j}")
                for c0, c1 in _exp_ranges(q0):
                    nc.scalar.activation(ep[:, c0:c1], psp[:, c0:c1], EXP,
                                         scale=SEXP)
                if d >= 0:
                    for h in range(2):
                        msl = slice(512 * h + q0, 512 * h + q0 + P)
                        nc.vector.tensor_mul(ep[:, msl], ep[:, msl], maskb[:])
                return ep

            def emit_pv(b, i, j, ep, po):
                # po[q, 0:64] += E^T(tile j) @ V(tile j); col 64 sums E
                d = j - 4 * i
                for t in range(max(0, d), 4):
                    for h in range(2):
                        s = h * 4 + t
                        nc.tensor.matmul(
                            po[:, s, 0:HD + 1],
                            ep[:, 512 * h + t * P:512 * h + (t + 1) * P],
                            vaug[b % 2][:, j, h, :],
                            start=(j == 0 and t == 0),
                            stop=(j == 4 * i + t))

            def emit_proj(k):
                if not wp_loaded:
                    # deferred so the 2MiB w_proj load doesn't delay startup
                    nc.sync.dma_start(wpb_sb[:], wpb[:])
                    nc.sync.dma_start(wprb_sb[:], wprb[:])
                    wp_loaded.append(True)
                yg = yg_pool.tile([P, NCORES, TPH], bf16, tag="yg",
                                  name=f"yg_{k}")
                nc.sync.dma_start(yg[:], g_out[k].rearrange("c p t -> p c t"))
                pp0 = shp_pool.tile([P, 512], f32, tag="shp", name=f"pp0_{k}")
                pp1 = shp_pool.tile([P, 512], f32, tag="shp", name=f"pp1_{k}")
                for ct in range(KT):
                    nc.tensor.matmul(pp0[:], yg[:, ct, :], wpb_sb[:, ct, 0:512],
                                     start=(ct == 0), stop=(ct == KT - 1))
                for ct in range(KT):
                    nc.tensor.matmul(pp1[:], yg[:, ct, :], wpb_sb[:, ct, 512:C],
                                     start=(ct == 0), stop=(ct == KT - 1))
                ob = ob_pool.tile([P, C], f32, tag="ob", name=f"ob_{k}")
                nc.vector.tensor_copy(ob[:, 0:512], pp0[:])
                nc.vector.tensor_copy(ob[:, 512:C], pp1[:])
                nc.sync.dma_start(yp[k // 2, k % 2, :, :], ob[:])

            def emit_proj_partial(yTh):
                # last half-batch: row-parallel partials over my 128 chans
                for tt in range(T // 2 // P):
                    pp0 = shp_pool.tile([P, 512], f32, tag="shp",
                                        name=f"lp0_{tt}")
                    pp1 = shp_pool.tile([P, 512], f32, tag="shp",
                                        name=f"lp1_{tt}")
                    nc.tensor.matmul(pp0[:], yTh[:, tt, :], wprb_sb[:, 0:512],
                                     start=True, stop=True)
                    nc.tensor.matmul(pp1[:], yTh[:, tt, :], wprb_sb[:, 512:C],
                                     start=True, stop=True)
                    obl = ob_pool.tile([P, C], bf16, tag="obl", name=f"obl_{tt}")
                    nc.vector.tensor_copy(obl[:, 0:512], pp0[:])
                    nc.vector.tensor_copy(obl[:, 512:C], pp1[:])
                    nc.sync.dma_start(ypl[tt * P:(tt + 1) * P, :], obl[:])

            def emit_exchange(k, yTh):
                nc.sync.dma_start(g_in[k].rearrange("j p t -> p j t"),
                                  yTh[:, :, :])
                nc.gpsimd.collective_compute(
                    "AllToAll",
                    mybir.AluOpType.bypass,
                    replica_groups=[list(range(NCORES))],
                    ins=[g_in[k][:]],
                    outs=[g_out[k][:]],
                )

            # deferred per-block finish: normalize -> transpose -> y^T slab
            # (+ exchange/proj), flushed a few key-tiles into the NEXT block
            # so the PE never sits behind the DVE/Pool normalize chain
            pending = []

            def make_blockend(b, i, posb, yT):
                def run():
                    y8b = y8b_pool.tile([P, 4, P], bf16, tag="y8b",
                                        name=f"y8b_{b}_{i}")
                    for t in range(4):
                        for h in range(2):
                            s = h * 4 + t
                            nc.gpsimd.normalize_recip(
                                y8b[:, t, HD * h:HD * h + HD],
                                posb[:, s, 0:HD],
                                posb[:, s, HD:HD + 1])
                    yTp = shp_pool.tile([P, 4, P], bf16, tag="shp",
                                        name=f"yTp_{b}_{i}")
                    for t in range(4):
                        nc.tensor.matmul(yTp[:, t, :], y8b[:, t, :], identb[:],
                                         is_transpose=True,
                                         start=(t == 0), stop=(t == 3))
                    nc.vector.tensor_copy(yT[:, (i % 2) * 4:(i % 2) * 4 + 4, :],
                                          yTp[:])
                    if i % 2 == 1:
                        k = 2 * b + i // 2
                        if k < 2 * B - 1:
                            emit_exchange(k, yT)
                            if k >= 3:
                                emit_proj(k - 3)
                        else:
                            for kk in range(2 * B - 4, 2 * B - 1):
                                emit_proj(kk)
                            emit_proj_partial(yT)
                return run

            emit_stage1(0)
            next_g = 1
            yT = None
            for b in range(B):
                for i in range(NQ):
                    nj = 4 * (i + 1)
                    if i % 2 == 0:
                        yT = yt_pool.tile([P, 8, TPH], bf16, tag="yT",
                                          name=f"yT_{b}_{i // 2}")
                    po = pop_pool.tile([P, 8, P], f32, tag="po",
                                       name=f"po_{b}_{i}")
                    eps = {}
                    depth = 3
                    for j in range(min(depth, nj)):
                        eps[j] = emit_s(b, i, j)
                    for j in range(nj):
                        if j + depth < nj:
                            eps[j + depth] = emit_s(b, i, j + depth)
                        if j == 1 and next_g < NTB:
                            emit_stage1(next_g)
                            next_g += 1
                        if j == 3 and pending:
                            pending.pop(0)()
                        emit_pv(b, i, j, eps.pop(j), po)
                    posb = posb_pool.tile([P, 8, HD + 1], f32, tag="posb",
                                          name=f"posb_{b}_{i}")
                    nc.vector.tensor_copy(posb[:], po[:, :, 0:HD + 1])
                    pending.append(make_blockend(b, i, posb, yT))
            while pending:
                pending.pop(0)()

    nc.compile()
    return nc


def _prep_inputs(x, w_attn, b_attn, w_proj):
    x = np.asarray(x, dtype=np.float32)
    w_attn = np.asarray(w_attn, dtype=np.float32)
    b_attn = np.asarray(b_attn, dtype=np.float32)
    w_proj = np.asarray(w_proj, dtype=np.float32)

    xT = np.ascontiguousarray(x.reshape(BT, C).T)          # [C, BT]
    # xp8[tb, p, k2, s2, c] = xT[k2*256 + s2*128 + p, tb*512 + c]
    xp8 = np.ascontiguousarray(
        xT.reshape(KT2, 2, P, NTB, 512).transpose(3, 2, 0, 1, 4)).astype(E4NP)
    # xpb[tb, p, kt, c] = xT[kt*128 + p, tb*512 + c]
    xpb = np.ascontiguousarray(
        xT.reshape(KT, P, NTB, 512).transpose(2, 1, 0, 3)).astype(BFNP)

    wpb = np.ascontiguousarray(
        w_proj.reshape(KT, P, C).transpose(1, 0, 2)).astype(BFNP)

    in_maps = []
    for c in range(NCORES):
        cols = slice(P * c, P * (c + 1))

        def wslice8(off):
            w = WS * w_attn[:, off + P * c: off + P * (c + 1)]  # [1024, 128]
            return np.ascontiguousarray(
                w.reshape(KT2, 2, P, P).transpose(2, 0, 1, 3)).astype(E4NP)

        wv = w_attn[:, 2 * C + P * c: 2 * C + P * (c + 1)]
        wvb = np.ascontiguousarray(
            wv.reshape(KT, P, P).transpose(1, 0, 2)).astype(BFNP)

        in_maps.append({
            "xp8": xp8,
            "xpb": xpb,
            "wq8": wslice8(0),
            "wk8": wslice8(C),
            "wvb": wvb,
            "wpb": wpb,
            "wprb": np.ascontiguousarray(w_proj[cols, :]).astype(BFNP),
            "bq": (WS * np.ascontiguousarray(b_attn[cols])).reshape(P, 1),
            "bk": (WS * np.ascontiguousarray(
                b_attn[C + P * c: C + P * (c + 1)])).reshape(P, 1),
        })
    return in_maps


def kernel(x, w_attn, b_attn, w_proj, b_proj):
    from concourse.bass_utils import run_bass_kernel_spmd

    if "nc" not in _CACHED:
        _CACHED["nc"] = _build_nc()
    nc = _CACHED["nc"]

    in_maps = _prep_inputs(x, w_attn, b_attn, w_proj)
    res = run_bass_kernel_spmd(nc, in_maps, core_ids=list(range(NCORES)))

    # core c holds tokens [h*1024 + c*128, +128) of each batch half h,
    # except the last half-batch which comes back as row-parallel partials
    y = np.empty((B, T, C), dtype=np.float32)
    for c in range(NCORES):
        part = res.results[c]["yp"]          # [B, 2, 128, C] f32
        for h in range(2):
            y[:, h * (T // 2) + c * 128: h * (T // 2) + (c + 1) * 128, :] = part[:, h]
    acc = res.results[0]["ypl"].astype(np.float32)
    for c in range(1, NCORES):
        acc = acc + res.results[c]["ypl"].astype(np.float32)
    y[B - 1, T // 2:, :] = acc
    # b_v folds into a constant output shift (softmax weights sum to 1)
    bias = np.asarray(b_proj, dtype=np.float32) + \
        np.asarray(b_attn, dtype=np.float32)[2 * C:] @ np.asarray(
            w_proj, dtype=np.float32)
    y += bias
    return y


# revision 8
# speedup vs baseline: 1.3183x; 1.0795x over previous
"""Causal self-attention (GPT-style block) on 8 Trainium2 NeuronCores.

Sharding: tensor-parallel over heads (16 heads / 8 cores = 2 per core),
c_attn column-parallel from the full input x, attention fully local per
core, c_proj token-parallel after an on-device AllToAll for batches 0-2
and row-parallel (host-summed partials) for batch 3.

Mixed precision (chosen against the 2e-2 gate by numpy simulation of
every quantization spot; measured 1.4e-2 end to end on the real data):
- Score path in fp8e4m3: q/k generation fp8 DoubleRow (w_qk pre-scaled
  x16 on the host for the fp8 subnormal floor, compensated in the exp
  scale), S = K^T.T @ Q^T fp8 DoubleRow with the 64-deep head
  contraction zero-padded in the second k-subtile (the cost model and
  PE charge by output rows only).
- Value path in bf16 (fp8 anywhere on it costs 2.6-3.7e-2): v
  generation emitted token-major (x^T tile stationary) so no PE
  transposes are needed, PV in fat-M orientation (out po[q,65] per key
  tile/head, 2.2x fewer PE rows than the 65-row-out orientation), bf16
  c_proj.
- exp is the only ACT work (~153us busy = the critical path); ep tiles
  are [128, 1024] (key tile x 2 heads) with diagonal tiles clipped at
  the 128-granular diagonal and ranges merged where an extra ACT
  instruction (~185ns) costs more than exp-ing dead columns.
- Normalize via gpsimd InstNormalizeRecip (division + bf16 cast in one
  Pool op); y^T via 4 PE transposes per block.

Scheduling (everything below is about keeping ACT 100% fed, because
exp is the roofline):
- PE work that is not S/PV (stage-1 qkv, c_proj units, row-parallel
  tail) is cut into <=2us closures on a filler queue and drained one
  per key tile, so the in-order PE stream never runs a long burst that
  starves exp of fresh S tiles (a 16-matmul proj burst = 11us ACT gap).
- Stage-1 of token block g+1 is pushed at the start of attention block
  g (double-buffered even/odd slabs), so batch boundaries don't drain
  ACT.
- AllToAll costs 21.5us in the model and the COLLECTIVE_CORES device
  serializes, so exchanges go out every ~2 blocks and proj(k) is
  drained two units later; the last two half-batches skip the
  collective entirely (row-parallel partials summed on the host) so
  the tail doesn't sit on a cold PE behind the last collective.
- The per-block normalize->transpose->y^T chain is deferred a few key
  tiles into the next block so the PE doesn't wait on the Pool/DVE
  chain.
- b_v folds into a host-side output shift (softmax weights sum to 1);
  b_q/b_k ride the stage-1 psum evictions.
"""

import numpy as np
import ml_dtypes

P = 128
B = 4
T = 2048
BT = B * T            # 8192 tokens
C = 1024
KT = C // P           # 8 contraction tiles of 128
KT2 = KT // 2         # 4 DoubleRow pairs
NTB = BT // 512       # 16 token blocks of 512
HD = 64               # head dim
NQ = T // 512         # 4 query blocks per batch
NCORES = 8
TPH = T // NCORES // 2  # 128 tokens per core per half-batch exchange
WS = 16.0             # host prescale on w_q/w_k (fp8 subnormal floor)
SEXP = 0.125 / (WS * WS)
NEXCH = 6             # units 0-5 exchange+proj; units 6,7 row-parallel

E4NP = ml_dtypes.float8_e4m3
BFNP = ml_dtypes.bfloat16

_CACHED = {}


def _exp_ranges(q0):
    # valid score cols per [tile j | 2 heads] psum tile; merged across
    # gaps where the dead rows cost less than an ACT instruction
    if q0 == 0:
        return [(0, 1024)]
    if q0 == 128:
        return [(q0, 1024)]
    return [(q0, 512), (512 + q0, 1024)]


def _build_nc():
    import concourse.mybir as mybir
    import concourse.tile as tile
    from concourse import bacc
    from concourse.masks import make_identity

    f32 = mybir.dt.float32
    bf16 = mybir.dt.bfloat16
    f8 = mybir.dt.float8e4
    EXP = mybir.ActivationFunctionType.Exp
    DR = mybir.MatmulPerfMode.DoubleRow

    nc = bacc.Bacc("TRN2", target_bir_lowering=False, debug=False,
                   num_devices=NCORES)

    xp8 = nc.dram_tensor("xp8", [NTB, P, KT2, 2, 512], f8, kind="ExternalInput")
    xpb = nc.dram_tensor("xpb", [NTB, P, KT, 512], bf16, kind="ExternalInput")
    wq8 = nc.dram_tensor("wq8", [P, KT2, 2, P], f8, kind="ExternalInput")
    wk8 = nc.dram_tensor("wk8", [P, KT2, 2, P], f8, kind="ExternalInput")
    wvb = nc.dram_tensor("wvb", [P, KT, P], bf16, kind="ExternalInput")
    wpb = nc.dram_tensor("wpb", [P, KT, C], bf16, kind="ExternalInput")
    wprb = nc.dram_tensor("wprb", [P, C], bf16, kind="ExternalInput")
    bq = nc.dram_tensor("bq", [P, 1], f32, kind="ExternalInput")
    bk = nc.dram_tensor("bk", [P, 1], f32, kind="ExternalInput")
    # units 0-5 (batches 0-2): fully-reduced rows for my token shard
    yp = nc.dram_tensor("yp", [3, 2, TPH, C], f32, kind="ExternalOutput")
    # batch 3: row-parallel partials over my 128 channels (host sums)
    ypl = nc.dram_tensor("ypl", [T, C], bf16, kind="ExternalOutput")

    with tile.TileContext(nc) as tc:
        with (
            tc.tile_pool(name="const", bufs=1) as const,
            tc.tile_pool(name="slab", bufs=1) as slab,
            tc.tile_pool(name="yt", bufs=2) as yt_pool,
            tc.tile_pool(name="x8", bufs=2) as x8_pool,
            tc.tile_pool(name="xb", bufs=2) as xb_pool,
            tc.tile_pool(name="e", bufs=6) as e_pool,
            tc.tile_pool(name="pb", bufs=2) as posb_pool,
            tc.tile_pool(name="y8", bufs=2) as y8b_pool,
            tc.tile_pool(name="yg", bufs=2) as yg_pool,
            tc.tile_pool(name="ob", bufs=2) as ob_pool,
            tc.tile_pool(name="dram", bufs=1, space="DRAM") as dram_pool,
            tc.tile_pool(name="pss", bufs=2, space="PSUM") as pss_pool,
            tc.tile_pool(name="shp", bufs=2, space="PSUM") as shp_pool,
            tc.tile_pool(name="pop", bufs=1, space="PSUM") as pop_pool,
        ):
            g_in = [dram_pool.tile([NCORES, P, TPH], bf16, name=f"g_in{k}",
                                   tag=f"g_in{k}") for k in range(NEXCH)]
            g_out = [dram_pool.tile([NCORES, P, TPH], bf16, name=f"g_out{k}",
                                    tag=f"g_out{k}") for k in range(NEXCH)]

            # --- constants / weights ---
            wq8_sb = const.tile([P, KT2, 2, P], f8)
            wk8_sb = const.tile([P, KT2, 2, P], f8)
            wvb_sb = const.tile([P, KT, P], bf16)
            wpb_sb = const.tile([P, KT, C], bf16)
            wprb_sb = const.tile([P, C], bf16)
            bq_sb = const.tile([P, 1], f32)
            bk_sb = const.tile([P, 1], f32)

            # stage-1 slabs, manually double-buffered by batch parity
            qT8 = [slab.tile([P, 2, T], f8, name=f"qT8_{e}", tag=f"qT8_{e}")
                   for e in range(2)]
            kT8 = [slab.tile([P, 2, T], f8, name=f"kT8_{e}", tag=f"kT8_{e}")
                   for e in range(2)]
            vaug = [slab.tile([P, NQ * 4, 2, HD + 1], bf16, name=f"vaug_{e}",
                              tag=f"vaug_{e}") for e in range(2)]

            # startup order matters: the DMA device is serial in the cost
            # model, so the first token block must beat the weight bulk
            xt8_0 = x8_pool.tile([P, KT2, 2, 512], f8, name="xt8_0", tag="xt8")
            xtb_0 = xb_pool.tile([P, KT, 512], bf16, name="xtb_0", tag="xtb")
            nc.sync.dma_start(xt8_0[:], xp8[0])
            nc.sync.dma_start(wq8_sb[:], wq8[:])
            nc.sync.dma_start(bq_sb[:], bq[:])
            nc.sync.dma_start(wk8_sb[:], wk8[:])
            nc.sync.dma_start(bk_sb[:], bk[:])
            nc.sync.dma_start(xtb_0[:], xpb[0])
            nc.sync.dma_start(wvb_sb[:], wvb[:])

            # zero second k-subtile of the even slabs on the (idle) DVE;
            # odd slabs + ones columns can trail on Pool
            nc.vector.memset(qT8[0][:, 1, :], 0.0)
            nc.vector.memset(kT8[0][:, 1, :], 0.0)
            nc.gpsimd.memset(vaug[0][:, :, :, HD:HD + 1], 1.0)
            nc.gpsimd.memset(qT8[1][:, 1, :], 0.0)
            nc.gpsimd.memset(kT8[1][:, 1, :], 0.0)
            nc.gpsimd.memset(vaug[1][:, :, :, HD:HD + 1], 1.0)

            identf = const.tile([P, P], f32)
            make_identity(nc, identf[:])
            identb = const.tile([P, P], bf16)
            nc.vector.tensor_copy(identb[:], identf[:])

            # mask[p, u] = 1.0 if u >= p else 0.0 (upper-right triangle)
            mask_f = const.tile([P, P], f32)
            nc.gpsimd.memset(mask_f[:], 1.0)
            nc.gpsimd.affine_select(
                out=mask_f[:],
                in_=mask_f[:],
                compare_op=mybir.AluOpType.is_ge,
                fill=0.0,
                base=0,
                pattern=[[1, P]],
                channel_multiplier=-1,
            )
            maskb = const.tile([P, P], bf16)
            nc.vector.tensor_copy(maskb[:], mask_f[:])

            wp_loaded = []
            # two filler queues: stage-1 chunks MUST fully drain before the
            # attention block that reads them emits its S tiles (the Tile
            # framework orders by emission, so a read emitted before its
            # writer reads stale SBUF); proj/partial chunks can drain any
            # time after their inputs' emission
            s1q = []
            pjq = []

            def drain_filler():
                if s1q:
                    s1q.pop(0)()
                elif pjq:
                    pjq.pop(0)()

            def drain_all():
                while s1q or pjq:
                    drain_filler()

            def push_stage1(g):
                b, lb = g // 4, g % 4
                sl = slice(lb * 512, (lb + 1) * 512)
                if g > 0:
                    xt8 = x8_pool.tile([P, KT2, 2, 512], f8, name=f"xt8_{g}",
                                       tag="xt8")
                    xtb = xb_pool.tile([P, KT, 512], bf16, name=f"xtb_{g}",
                                       tag="xtb")
                    nc.sync.dma_start(xt8[:], xp8[g])
                    nc.sync.dma_start(xtb[:], xpb[g])
                else:
                    xt8, xtb = xt8_0, xtb_0

                def qk_chunk(w_sb, b_sb, dst):
                    def run():
                        ps = shp_pool.tile([P, 512], f32, tag="shp",
                                           name=f"ps_{g}")
                        for k2 in range(KT2):
                            nc.tensor.matmul(ps[:], w_sb[:, k2], xt8[:, k2],
                                             start=(k2 == 0),
                                             stop=(k2 == KT2 - 1),
                                             perf_mode=DR)
                        nc.vector.tensor_scalar_add(dst[:, 0, sl], ps[:],
                                                    b_sb[:])
                    return run

                def v_chunk(half):
                    # token-major v (x^T tile stationary); single psum
                    # bank per chunk, one start — zero-region covers the
                    # second tt slot
                    def run():
                        vps = shp_pool.tile([P, 2, 2, HD], f32, tag="shp",
                                            name=f"vps_{g}_{half}")
                        for tt2 in range(2):
                            tt = half * 2 + tt2
                            for kt in range(KT):
                                nc.tensor.matmul(
                                    vps[:, tt2],
                                    xtb[:, kt, tt * P:(tt + 1) * P],
                                    wvb_sb[:, kt, :],
                                    start=(tt2 == 0 and kt == 0),
                                    stop=(tt2 == 1 and kt == KT - 1))
                        j0 = lb * 4 + half * 2
                        nc.vector.tensor_copy(
                            vaug[b % 2][:, j0:j0 + 2, :, 0:HD], vps[:])
                    return run

                s1q.append(qk_chunk(wq8_sb, bq_sb, qT8[b % 2]))
                s1q.append(qk_chunk(wk8_sb, bk_sb, kT8[b % 2]))
                s1q.append(v_chunk(0))
                s1q.append(v_chunk(1))

            def push_proj(k):
                # exchange-path c_proj unit k, split into 4 psum-transient
                # matmul chunks + a final DMA chunk
                if not wp_loaded:
                    for kt in range(KT):
                        nc.sync.dma_start(wpb_sb[:, kt], wpb[:, kt])
                    nc.sync.dma_start(wprb_sb[:], wprb[:])
                    wp_loaded.append(True)
                state = {}

                def head():
                    yg = yg_pool.tile([P, NCORES, TPH], bf16, tag="yg",
                                      name=f"yg_{k}")
                    nc.sync.dma_start(yg[:],
                                      g_out[k].rearrange("c p t -> p c t"))
                    state["yg"] = yg
                    state["ob"] = ob_pool.tile([P, C], f32, tag="ob",
                                               name=f"ob_{k}")

                def mm_chunk(cc):
                    def run():
                        yg, ob = state["yg"], state["ob"]
                        csl = slice(cc * 256, (cc + 1) * 256)
                        pp = shp_pool.tile([P, 256], f32, tag="shp",
                                           name=f"pp_{k}_{cc}")
                        for ct in range(KT):
                            nc.tensor.matmul(pp[:], yg[:, ct, :],
                                             wpb_sb[:, ct, csl],
                                             start=(ct == 0),
                                             stop=(ct == KT - 1))
                        nc.vector.tensor_copy(ob[:, csl], pp[:])
                    return run

                def finish():
                    nc.sync.dma_start(yp[k // 2, k % 2, :, :], state["ob"])

                pjq.append(head)
                for cc in range(4):
                    pjq.append(mm_chunk(cc))
                pjq.append(finish)

            def push_partial(k, yTh):
                # row-parallel tail unit (no collective): my 128 channels
                # x full w_proj rows -> bf16 partials, host sums
                r0 = (k - NEXCH) * (T // 2)
                for tt in range(8):
                    def run(tt=tt):
                        pp0 = shp_pool.tile([P, 512], f32, tag="shp",
                                            name=f"lp0_{k}_{tt}")
                        pp1 = shp_pool.tile([P, 512], f32, tag="shp",
                                            name=f"lp1_{k}_{tt}")
                        nc.tensor.matmul(pp0[:], yTh[:, tt, :],
                                         wprb_sb[:, 0:512],
                                         start=True, stop=True)
                        nc.tensor.matmul(pp1[:], yTh[:, tt, :],
                                         wprb_sb[:, 512:C],
                                         start=True, stop=True)
                        obl = ob_pool.tile([P, C], bf16, tag="obl",
                                           name=f"obl_{k}_{tt}")
                        nc.vector.tensor_copy(obl[:, 0:512], pp0[:])
                        nc.vector.tensor_copy(obl[:, 512:C], pp1[:])
                        nc.sync.dma_start(
                            ypl[r0 + tt * P:r0 + (tt + 1) * P, :], obl[:])
                    pjq.append(run)

            def emit_s(b, i, j):
                # S^T[key, query] for both heads of tile j, fp8 DoubleRow
                d = j - 4 * i
                q0 = max(0, d) * P
                qb_, kb_ = qT8[b % 2], kT8[b % 2]
                psp = pss_pool.tile([P, 1024], f32, tag="pss",
                                    name=f"psp_{b}_{i}_{j}")
                for h in range(2):
                    nc.tensor.matmul(
                        psp[:, 512 * h + q0:512 * h + 512],
                        kb_[HD * h:HD * h + HD, :, j * P:(j + 1) * P],
                        qb_[HD * h:HD * h + HD, :, i * 512 + q0:(i + 1) * 512],
                        start=True, stop=True, perf_mode=DR,
                        tile_position=(HD * h, 0))
                ep = e_pool.tile([P, 1024], bf16, tag="e",
                                 name=f"ep_{b}_{i}_{j}")
                for c0, c1 in _exp_ranges(q0):
                    nc.scalar.activation(ep[:, c0:c1], psp[:, c0:c1], EXP,
                                         scale=SEXP)
                if d >= 0:
                    for h in range(2):
                        msl = slice(512 * h + q0, 512 * h + q0 + P)
                        nc.vector.tensor_mul(ep[:, msl], ep[:, msl], maskb[:])
                return ep

            def emit_pv(b, i, j, ep, po):
                # po[q, 0:64] += E^T(tile j) @ V(tile j); col 64 sums E;
                # single start per psum bank (zero-region covers slots)
                d = j - 4 * i
                for t in range(max(0, d), 4):
                    for h in range(2):
                        nc.tensor.matmul(
                            po[:, h * 4 + t, 0:HD + 1],
                            ep[:, 512 * h + t * P:512 * h + (t + 1) * P],
                            vaug[b % 2][:, j, h, :],
                            start=(j == 0 and t == 0),
                            stop=(j == 4 * i + t))

            def emit_exchange(k, yTh):
                nc.sync.dma_start(g_in[k].rearrange("j p t -> p j t"),
                                  yTh[:, :, :])
                nc.gpsimd.collective_compute(
                    "AllToAll",
                    mybir.AluOpType.bypass,
                    replica_groups=[list(range(NCORES))],
                    ins=[g_in[k][:]],
                    outs=[g_out[k][:]],
                )

            pending = []

            def make_blockend(b, i, posb, yT):
                def run():
                    y8b = y8b_pool.tile([P, 4, P], bf16, tag="y8b",
                                        name=f"y8b_{b}_{i}")
                    for t in range(4):
                        for h in range(2):
                            s = h * 4 + t
                            nc.gpsimd.normalize_recip(
                                y8b[:, t, HD * h:HD * h + HD],
                                posb[:, s, 0:HD],
                                posb[:, s, HD:HD + 1])
                    yTp = shp_pool.tile([P, 4, P], bf16, tag="shp",
                                        name=f"yTp_{b}_{i}")
                    for t in range(4):
                        nc.tensor.matmul(yTp[:, t, :], y8b[:, t, :], identb[:],
                                         is_transpose=True,
                                         start=(t == 0), stop=(t == 3))
                    nc.vector.tensor_copy(yT[:, (i % 2) * 4:(i % 2) * 4 + 4, :],
                                          yTp[:])
                    if i % 2 == 1:
                        k = 2 * b + i // 2
                        if k < NEXCH:
                            emit_exchange(k, yT)
                            if k >= 2:
                                push_proj(k - 2)
                        elif k == NEXCH:
                            push_partial(k, yT)
                            push_proj(NEXCH - 2)
                            push_proj(NEXCH - 1)
                        else:
                            drain_all()
                            push_partial(k, yT)
                            drain_all()
                return run

            push_stage1(0)
            drain_all()          # stage-1 of block 0 runs inline up front
            next_g = 1
            yT = None
            for b in range(B):
                for i in range(NQ):
                    nj = 4 * (i + 1)
                    while s1q:   # this block's q/k/v must be emitted first
                        s1q.pop(0)()
                    if next_g < NTB:
                        push_stage1(next_g)
                        next_g += 1
                    if i % 2 == 0:
                        yT = yt_pool.tile([P, 8, TPH], bf16, tag="yT",
                                          name=f"yT_{b}_{i // 2}")
                    po = pop_pool.tile([P, 8, P], f32, tag="po",
                                       name=f"po_{b}_{i}")
                    eps = {}
                    depth = 2
                    for j in range(min(depth, nj)):
                        eps[j] = emit_s(b, i, j)
                    for j in range(nj):
                        if j + depth < nj:
                            eps[j + depth] = emit_s(b, i, j + depth)
                        if j == 3 and pending:
                            pending.pop(0)()
                        drain_filler()
                        emit_pv(b, i, j, eps.pop(j), po)
                    posb = posb_pool.tile([P, 8, HD + 1], f32, tag="posb",
                                          name=f"posb_{b}_{i}")
                    nc.vector.tensor_copy(posb[:], po[:, :, 0:HD + 1])
                    pending.append(make_blockend(b, i, posb, yT))
            while pending:
                pending.pop(0)()
            drain_all()

    nc.compile()
    return nc


def _prep_inputs(x, w_attn, b_attn, w_proj):
    x = np.asarray(x, dtype=np.float32)
    w_attn = np.asarray(w_attn, dtype=np.float32)
    b_attn = np.asarray(b_attn, dtype=np.float32)
    w_proj = np.asarray(w_proj, dtype=np.float32)

    xT = np.ascontiguousarray(x.reshape(BT, C).T)          # [C, BT]
    # xp8[tb, p, k2, s2, c] = xT[k2*256 + s2*128 + p, tb*512 + c]
    xp8 = np.ascontiguousarray(
        xT.reshape(KT2, 2, P, NTB, 512).transpose(3, 2, 0, 1, 4)).astype(E4NP)
    # xpb[tb, p, kt, c] = xT[kt*128 + p, tb*512 + c]
    xpb = np.ascontiguousarray(
        xT.reshape(KT, P, NTB, 512).transpose(2, 1, 0, 3)).astype(BFNP)

    wpb = np.ascontiguousarray(
        w_proj.reshape(KT, P, C).transpose(1, 0, 2)).astype(BFNP)

    in_maps = []
    for c in range(NCORES):
        cols = slice(P * c, P * (c + 1))

        def wslice8(off):
            w = WS * w_attn[:, off + P * c: off + P * (c + 1)]  # [1024, 128]
            return np.ascontiguousarray(
                w.reshape(KT2, 2, P, P).transpose(2, 0, 1, 3)).astype(E4NP)

        wv = w_attn[:, 2 * C + P * c: 2 * C + P * (c + 1)]
        wvb = np.ascontiguousarray(
            wv.reshape(KT, P, P).transpose(1, 0, 2)).astype(BFNP)

        in_maps.append({
            "xp8": xp8,
            "xpb": xpb,
            "wq8": wslice8(0),
            "wk8": wslice8(C),
            "wvb": wvb,
            "wpb": wpb,
            "wprb": np.ascontiguousarray(w_proj[cols, :]).astype(BFNP),
            "bq": (WS * np.ascontiguousarray(b_attn[cols])).reshape(P, 1),
            "bk": (WS * np.ascontiguousarray(
                b_attn[C + P * c: C + P * (c + 1)])).reshape(P, 1),
        })
    return in_maps


def kernel(x, w_attn, b_attn, w_proj, b_proj):
    from concourse.bass_utils import run_bass_kernel_spmd

    if "nc" not in _CACHED:
        _CACHED["nc"] = _build_nc()
    nc = _CACHED["nc"]

    in_maps = _prep_inputs(x, w_attn, b_attn, w_proj)
    res = run_bass_kernel_spmd(nc, in_maps, core_ids=list(range(NCORES)))

    # batches 0-2: core c holds tokens [h*1024 + c*128, +128) of each
    # half h; batch 3 comes back as row-parallel partials (bf16)
    y = np.empty((B, T, C), dtype=np.float32)
    for c in range(NCORES):
        part = res.results[c]["yp"]          # [3, 2, 128, C] f32
        for h in range(2):
            y[:3, h * (T // 2) + c * 128: h * (T // 2) + (c + 1) * 128, :] = \
                part[:, h]
    acc = res.results[0]["ypl"].astype(np.float32)
    for c in range(1, NCORES):
        acc = acc + res.results[c]["ypl"].astype(np.float32)
    y[B - 1] = acc
    # b_v folds into a constant output shift (softmax weights sum to 1)
    bias = np.asarray(b_proj, dtype=np.float32) + \
        np.asarray(b_attn, dtype=np.float32)[2 * C:] @ np.asarray(
            w_proj, dtype=np.float32)
    y += bias
    return y


# revision 9
# speedup vs baseline: 1.3952x; 1.0583x over previous
"""Causal self-attention (GPT-style block) on 8 Trainium2 NeuronCores.

Sharding: tensor-parallel over heads (16 heads / 8 cores = 2 per core),
c_attn column-parallel from the full input x, attention fully local per
core, c_proj token-parallel after an on-device AllToAll for batches 0-2
and row-parallel (host-summed partials) for batch 3.

Mixed precision (chosen against the 2e-2 gate by numpy simulation of
every quantization spot; measured 1.4e-2 end to end on the real data):
- Score path in fp8e4m3: q/k generation fp8 DoubleRow (w_qk pre-scaled
  x16 on the host for the fp8 subnormal floor, compensated in the exp
  scale), S = K^T.T @ Q^T fp8 DoubleRow with the 64-deep head
  contraction zero-padded in the second k-subtile (the cost model and
  PE charge by output rows only).
- Value path in bf16 (fp8 anywhere on it costs 2.6-3.7e-2): v
  generation emitted token-major (x^T tile stationary) so no PE
  transposes are needed, PV in fat-M orientation (out po[q,65] per key
  tile/head, 2.2x fewer PE rows than the 65-row-out orientation), bf16
  c_proj.
- exp is the only ACT work (~153us busy = the critical path); ep tiles
  are [128, 1024] (key tile x 2 heads) with diagonal tiles clipped at
  the 128-granular diagonal and ranges merged where an extra ACT
  instruction (~185ns) costs more than exp-ing dead columns.
- Normalize via gpsimd InstNormalizeRecip (division + bf16 cast in one
  Pool op); y^T via 4 PE transposes per block.

Scheduling (everything below is about keeping ACT 100% fed, because
exp is the roofline):
- PE work that is not S/PV (stage-1 qkv, c_proj units, row-parallel
  tail) is cut into <=2us closures on a filler queue and drained one
  per key tile, so the in-order PE stream never runs a long burst that
  starves exp of fresh S tiles (a 16-matmul proj burst = 11us ACT gap).
- Stage-1 of token block g+1 is pushed at the start of attention block
  g (double-buffered even/odd slabs), so batch boundaries don't drain
  ACT.
- AllToAll costs 21.5us in the model and the COLLECTIVE_CORES device
  serializes, so exchanges go out every ~2 blocks and proj(k) is
  drained two units later; the last two half-batches skip the
  collective entirely (row-parallel partials summed on the host) so
  the tail doesn't sit on a cold PE behind the last collective.
- The per-block normalize->transpose->y^T chain is deferred a few key
  tiles into the next block so the PE doesn't wait on the Pool/DVE
  chain.
- b_v folds into a host-side output shift (softmax weights sum to 1);
  b_q/b_k ride the stage-1 psum evictions.
"""

import numpy as np
import ml_dtypes

P = 128
B = 4
T = 2048
BT = B * T            # 8192 tokens
C = 1024
KT = C // P           # 8 contraction tiles of 128
KT2 = KT // 2         # 4 DoubleRow pairs
NTB = BT // 512       # 16 token blocks of 512
HD = 64               # head dim
NQ = T // 512         # 4 query blocks per batch
NCORES = 8
TPH = T // NCORES // 2  # 128 tokens per core per half-batch exchange
WS = 16.0             # host prescale on w_q/w_k (fp8 subnormal floor)
SEXP = 0.125 / (WS * WS)
NEXCH = 6             # units 0-5 exchange+proj; units 6,7 row-parallel

E4NP = ml_dtypes.float8_e4m3
BFNP = ml_dtypes.bfloat16

_CACHED = {}


def _exp_ranges(q0):
    # valid score cols per [tile j | 2 heads] psum tile; merged across
    # gaps where the dead rows cost less than an ACT instruction
    if q0 == 0:
        return [(0, 1024)]
    if q0 == 128:
        return [(q0, 1024)]
    return [(q0, 512), (512 + q0, 1024)]


def _build_nc():
    import concourse.mybir as mybir
    import concourse.tile as tile
    from concourse import bacc
    from concourse.masks import make_identity

    f32 = mybir.dt.float32
    bf16 = mybir.dt.bfloat16
    f8 = mybir.dt.float8e4
    EXP = mybir.ActivationFunctionType.Exp
    DR = mybir.MatmulPerfMode.DoubleRow

    nc = bacc.Bacc("TRN2", target_bir_lowering=False, debug=False,
                   num_devices=NCORES)

    xp8 = nc.dram_tensor("xp8", [NTB, P, KT2, 2, 512], f8, kind="ExternalInput")
    xpb = nc.dram_tensor("xpb", [NTB, P, KT, 512], bf16, kind="ExternalInput")
    wq8 = nc.dram_tensor("wq8", [P, KT2, 2, P], f8, kind="ExternalInput")
    wk8 = nc.dram_tensor("wk8", [P, KT2, 2, P], f8, kind="ExternalInput")
    wvb = nc.dram_tensor("wvb", [P, KT, P], bf16, kind="ExternalInput")
    wpb = nc.dram_tensor("wpb", [P, KT, C], bf16, kind="ExternalInput")
    wprb = nc.dram_tensor("wprb", [P, C], bf16, kind="ExternalInput")
    bq = nc.dram_tensor("bq", [P, 1], f32, kind="ExternalInput")
    bk = nc.dram_tensor("bk", [P, 1], f32, kind="ExternalInput")
    # units 0-5 (batches 0-2): fully-reduced rows for my token shard
    yp = nc.dram_tensor("yp", [3, 2, TPH, C], f32, kind="ExternalOutput")
    # batch 3: row-parallel partials over my 128 channels (host sums)
    ypl = nc.dram_tensor("ypl", [T, C], bf16, kind="ExternalOutput")

    with tile.TileContext(nc) as tc:
        with (
            tc.tile_pool(name="const", bufs=1) as const,
            tc.tile_pool(name="slab", bufs=1) as slab,
            tc.tile_pool(name="yt", bufs=2) as yt_pool,
            tc.tile_pool(name="x8", bufs=2) as x8_pool,
            tc.tile_pool(name="xb", bufs=2) as xb_pool,
            tc.tile_pool(name="e", bufs=6) as e_pool,
            tc.tile_pool(name="pb", bufs=2) as posb_pool,
            tc.tile_pool(name="y8", bufs=2) as y8b_pool,
            tc.tile_pool(name="yg", bufs=2) as yg_pool,
            tc.tile_pool(name="ob", bufs=2) as ob_pool,
            tc.tile_pool(name="dram", bufs=1, space="DRAM") as dram_pool,
            tc.tile_pool(name="pss", bufs=2, space="PSUM") as pss_pool,
            tc.tile_pool(name="shp", bufs=2, space="PSUM") as shp_pool,
            tc.tile_pool(name="pop", bufs=1, space="PSUM") as pop_pool,
        ):
            g_in = [dram_pool.tile([NCORES, P, TPH], bf16, name=f"g_in{k}",
                                   tag=f"g_in{k}") for k in range(NEXCH)]
            g_out = [dram_pool.tile([NCORES, P, TPH], bf16, name=f"g_out{k}",
                                    tag=f"g_out{k}") for k in range(NEXCH)]

            # --- constants / weights ---
            wq8_sb = const.tile([P, KT2, 2, P], f8)
            wk8_sb = const.tile([P, KT2, 2, P], f8)
            wvb_sb = const.tile([P, KT, P], bf16)
            wpb_sb = const.tile([P, KT, C], bf16)
            wprb_sb = const.tile([P, C], bf16)
            bq_sb = const.tile([P, 1], f32)
            bk_sb = const.tile([P, 1], f32)

            # stage-1 slabs, manually double-buffered by batch parity
            qT8 = [slab.tile([P, 2, T], f8, name=f"qT8_{e}", tag=f"qT8_{e}")
                   for e in range(2)]
            kT8 = [slab.tile([P, 2, T], f8, name=f"kT8_{e}", tag=f"kT8_{e}")
                   for e in range(2)]
            vaug = [slab.tile([P, NQ * 4, 2, HD + 1], bf16, name=f"vaug_{e}",
                              tag=f"vaug_{e}") for e in range(2)]

            # startup order matters: the DMA device is serial in the cost
            # model, so the first token block must beat the weight bulk
            xt8_0 = x8_pool.tile([P, KT2, 2, 512], f8, name="xt8_0", tag="xt8")
            xtb_0 = xb_pool.tile([P, KT, 512], bf16, name="xtb_0", tag="xtb")
            nc.sync.dma_start(xt8_0[:], xp8[0])
            nc.sync.dma_start(wq8_sb[:], wq8[:])
            nc.sync.dma_start(bq_sb[:], bq[:])
            nc.sync.dma_start(wk8_sb[:], wk8[:])
            nc.sync.dma_start(bk_sb[:], bk[:])
            nc.sync.dma_start(xtb_0[:], xpb[0])
            nc.sync.dma_start(wvb_sb[:], wvb[:])

            # zero second k-subtile of the even slabs on the (idle) DVE;
            # odd slabs + ones columns can trail on Pool
            nc.vector.memset(qT8[0][:, 1, :], 0.0)
            nc.vector.memset(kT8[0][:, 1, :], 0.0)
            nc.gpsimd.memset(vaug[0][:, :, :, HD:HD + 1], 1.0)
            nc.gpsimd.memset(qT8[1][:, 1, :], 0.0)
            nc.gpsimd.memset(kT8[1][:, 1, :], 0.0)
            nc.gpsimd.memset(vaug[1][:, :, :, HD:HD + 1], 1.0)

            identf = const.tile([P, P], f32)
            make_identity(nc, identf[:])
            identb = const.tile([P, P], bf16)
            nc.vector.tensor_copy(identb[:], identf[:])

            # mask[p, u] = 1.0 if u >= p else 0.0 (upper-right triangle)
            mask_f = const.tile([P, P], f32)
            nc.gpsimd.memset(mask_f[:], 1.0)
            nc.gpsimd.affine_select(
                out=mask_f[:],
                in_=mask_f[:],
                compare_op=mybir.AluOpType.is_ge,
                fill=0.0,
                base=0,
                pattern=[[1, P]],
                channel_multiplier=-1,
            )
            maskb = const.tile([P, P], bf16)
            nc.vector.tensor_copy(maskb[:], mask_f[:])

            wp_loaded = []
            # two filler queues: stage-1 chunks MUST fully drain before the
            # attention block that reads them emits its S tiles (the Tile
            # framework orders by emission, so a read emitted before its
            # writer reads stale SBUF); proj/partial chunks can drain any
            # time after their inputs' emission
            s1q = []
            pjq = []

            def drain_filler():
                if s1q:
                    s1q.pop(0)()
                elif pjq:
                    pjq.pop(0)()

            def drain_all():
                while s1q or pjq:
                    drain_filler()

            def push_stage1(g):
                b, lb = g // 4, g % 4
                sl = slice(lb * 512, (lb + 1) * 512)
                if g > 0:
                    xt8 = x8_pool.tile([P, KT2, 2, 512], f8, name=f"xt8_{g}",
                                       tag="xt8")
                    xtb = xb_pool.tile([P, KT, 512], bf16, name=f"xtb_{g}",
                                       tag="xtb")
                    nc.sync.dma_start(xt8[:], xp8[g])
                    nc.sync.dma_start(xtb[:], xpb[g])
                else:
                    xt8, xtb = xt8_0, xtb_0

                def qk_chunk(w_sb, b_sb, dst):
                    def run():
                        ps = shp_pool.tile([P, 512], f32, tag="shp",
                                           name=f"ps_{g}")
                        for k2 in range(KT2):
                            nc.tensor.matmul(ps[:], w_sb[:, k2], xt8[:, k2],
                                             start=(k2 == 0),
                                             stop=(k2 == KT2 - 1),
                                             perf_mode=DR)
                        nc.vector.tensor_scalar_add(dst[:, 0, sl], ps[:],
                                                    b_sb[:])
                    return run

                def v_chunk(half):
                    # token-major v (x^T tile stationary); single psum
                    # bank per chunk, one start — zero-region covers the
                    # second tt slot
                    def run():
                        vps = shp_pool.tile([P, 2, 2, HD], f32, tag="shp",
                                            name=f"vps_{g}_{half}")
                        for tt2 in range(2):
                            tt = half * 2 + tt2
                            for kt in range(KT):
                                nc.tensor.matmul(
                                    vps[:, tt2],
                                    xtb[:, kt, tt * P:(tt + 1) * P],
                                    wvb_sb[:, kt, :],
                                    start=(tt2 == 0 and kt == 0),
                                    stop=(tt2 == 1 and kt == KT - 1))
                        j0 = lb * 4 + half * 2
                        nc.vector.tensor_copy(
                            vaug[b % 2][:, j0:j0 + 2, :, 0:HD], vps[:])
                    return run

                s1q.append(qk_chunk(wq8_sb, bq_sb, qT8[b % 2]))
                s1q.append(qk_chunk(wk8_sb, bk_sb, kT8[b % 2]))
                s1q.append(v_chunk(0))
                s1q.append(v_chunk(1))

            proj_state = {}

            def start_proj(k):
                # issue the gathered-y load for unit k NOW, on the Pool
                # queue: it waits on collective k without head-of-line
                # blocking the x-stream DMAs on the sync queue
                if not wp_loaded:
                    for kt in range(KT):
                        nc.sync.dma_start(wpb_sb[:, kt], wpb[:, kt])
                    nc.sync.dma_start(wprb_sb[:], wprb[:])
                    wp_loaded.append(true := True)
                yg = yg_pool.tile([P, NCORES, TPH], bf16, tag="yg",
                                  name=f"yg_{k}")
                nc.gpsimd.dma_start(yg[:], g_out[k].rearrange("c p t -> p c t"))
                proj_state[k] = yg

            def push_proj(k):
                # exchange-path c_proj unit k: 4 psum-transient matmul
                # chunks + a final DMA chunk (yg was loaded by start_proj)
                state = {}

                def head():
                    state["ob"] = ob_pool.tile([P, C], f32, tag="ob",
                                               name=f"ob_{k}")

                def mm_chunk(cc):
                    def run():
                        yg, ob = proj_state[k], state["ob"]
                        csl = slice(cc * 256, (cc + 1) * 256)
                        pp = shp_pool.tile([P, 256], f32, tag="shp",
                                           name=f"pp_{k}_{cc}")
                        for ct in range(KT):
                            nc.tensor.matmul(pp[:], yg[:, ct, :],
                                             wpb_sb[:, ct, csl],
                                             start=(ct == 0),
                                             stop=(ct == KT - 1))
                        nc.vector.tensor_copy(ob[:, csl], pp[:])
                    return run

                def finish():
                    nc.sync.dma_start(yp[k // 2, k % 2, :, :], state["ob"])

                pjq.append(head)
                for cc in range(4):
                    pjq.append(mm_chunk(cc))
                pjq.append(finish)

            def push_partial(k, yTh):
                # row-parallel tail unit (no collective): my 128 channels
                # x full w_proj rows -> bf16 partials, host sums
                r0 = (k - NEXCH) * (T // 2)
                for tt in range(8):
                    def run(tt=tt):
                        pp0 = shp_pool.tile([P, 512], f32, tag="shp",
                                            name=f"lp0_{k}_{tt}")
                        pp1 = shp_pool.tile([P, 512], f32, tag="shp",
                                            name=f"lp1_{k}_{tt}")
                        nc.tensor.matmul(pp0[:], yTh[:, tt, :],
                                         wprb_sb[:, 0:512],
                                         start=True, stop=True)
                        nc.tensor.matmul(pp1[:], yTh[:, tt, :],
                                         wprb_sb[:, 512:C],
                                         start=True, stop=True)
                        obl = ob_pool.tile([P, C], bf16, tag="obl",
                                           name=f"obl_{k}_{tt}")
                        nc.vector.tensor_copy(obl[:, 0:512], pp0[:])
                        nc.vector.tensor_copy(obl[:, 512:C], pp1[:])
                        nc.sync.dma_start(
                            ypl[r0 + tt * P:r0 + (tt + 1) * P, :], obl[:])
                    pjq.append(run)

            def emit_s(b, i, j):
                # S^T[key, query] for both heads of tile j, fp8 DoubleRow
                d = j - 4 * i
                q0 = max(0, d) * P
                qb_, kb_ = qT8[b % 2], kT8[b % 2]
                psp = pss_pool.tile([P, 1024], f32, tag="pss",
                                    name=f"psp_{b}_{i}_{j}")
                for h in range(2):
                    nc.tensor.matmul(
                        psp[:, 512 * h + q0:512 * h + 512],
                        kb_[HD * h:HD * h + HD, :, j * P:(j + 1) * P],
                        qb_[HD * h:HD * h + HD, :, i * 512 + q0:(i + 1) * 512],
                        start=True, stop=True, perf_mode=DR,
                        tile_position=(HD * h, 0))
                ep = e_pool.tile([P, 1024], bf16, tag="e",
                                 name=f"ep_{b}_{i}_{j}")
                for c0, c1 in _exp_ranges(q0):
                    nc.scalar.activation(ep[:, c0:c1], psp[:, c0:c1], EXP,
                                         scale=SEXP)
                if d >= 0:
                    for h in range(2):
                        msl = slice(512 * h + q0, 512 * h + q0 + P)
                        nc.vector.tensor_mul(ep[:, msl], ep[:, msl], maskb[:])
                return ep

            def emit_pv(b, i, j, ep, po):
                # po[q, 0:64] += E^T(tile j) @ V(tile j); col 64 sums E;
                # single start per psum bank (zero-region covers slots)
                d = j - 4 * i
                for t in range(max(0, d), 4):
                    for h in range(2):
                        nc.tensor.matmul(
                            po[:, h * 4 + t, 0:HD + 1],
                            ep[:, 512 * h + t * P:512 * h + (t + 1) * P],
                            vaug[b % 2][:, j, h, :],
                            start=(j == 0 and t == 0),
                            stop=(j == 4 * i + t))

            def emit_exchange(k, yTh):
                nc.sync.dma_start(g_in[k].rearrange("j p t -> p j t"),
                                  yTh[:, :, :])
                nc.gpsimd.collective_compute(
                    "AllToAll",
                    mybir.AluOpType.bypass,
                    replica_groups=[list(range(NCORES))],
                    ins=[g_in[k][:]],
                    outs=[g_out[k][:]],
                )

            pending = []

            def make_blockend(b, i, posb, yT):
                def run():
                    y8b = y8b_pool.tile([P, 4, P], bf16, tag="y8b",
                                        name=f"y8b_{b}_{i}")
                    for t in range(4):
                        for h in range(2):
                            s = h * 4 + t
                            nc.gpsimd.normalize_recip(
                                y8b[:, t, HD * h:HD * h + HD],
                                posb[:, s, 0:HD],
                                posb[:, s, HD:HD + 1])
                    yTp = shp_pool.tile([P, 4, P], bf16, tag="shp",
                                        name=f"yTp_{b}_{i}")
                    for t in range(4):
                        nc.tensor.matmul(yTp[:, t, :], y8b[:, t, :], identb[:],
                                         is_transpose=True,
                                         start=(t == 0), stop=(t == 3))
                    nc.vector.tensor_copy(yT[:, (i % 2) * 4:(i % 2) * 4 + 4, :],
                                          yTp[:])
                    if i % 2 == 1:
                        k = 2 * b + i // 2
                        if k < NEXCH:
                            emit_exchange(k, yT)
                            if k >= 1:
                                start_proj(k - 1)
                            if k >= 2:
                                push_proj(k - 2)
                        elif k == NEXCH:
                            start_proj(NEXCH - 1)
                            push_proj(NEXCH - 2)
                            push_partial(k, yT)
                            push_proj(NEXCH - 1)
                        else:
                            drain_all()
                            push_partial(k, yT)
                            drain_all()
                return run

            push_stage1(0)
            drain_all()          # stage-1 of block 0 runs inline up front
            next_g = 1
            yT = None
            for b in range(B):
                for i in range(NQ):
                    nj = 4 * (i + 1)
                    while s1q:   # this block's q/k/v must be emitted first
                        s1q.pop(0)()
                    if next_g < NTB:
                        push_stage1(next_g)
                        next_g += 1
                    if i % 2 == 0:
                        yT = yt_pool.tile([P, 8, TPH], bf16, tag="yT",
                                          name=f"yT_{b}_{i // 2}")
                    po = pop_pool.tile([P, 8, P], f32, tag="po",
                                       name=f"po_{b}_{i}")
                    eps = {}
                    depth = 2
                    for j in range(min(depth, nj)):
                        eps[j] = emit_s(b, i, j)
                    for j in range(nj):
                        if j + depth < nj:
                            eps[j + depth] = emit_s(b, i, j + depth)
                        if j == 3 and pending:
                            pending.pop(0)()
                        drain_filler()
                        emit_pv(b, i, j, eps.pop(j), po)
                    posb = posb_pool.tile([P, 8, HD + 1], f32, tag="posb",
                                          name=f"posb_{b}_{i}")
                    nc.vector.tensor_copy(posb[:], po[:, :, 0:HD + 1])
                    pending.append(make_blockend(b, i, posb, yT))
            while pending:
                pending.pop(0)()
            drain_all()

    nc.compile()
    return nc


def _prep_inputs(x, w_attn, b_attn, w_proj):
    x = np.asarray(x, dtype=np.float32)
    w_attn = np.asarray(w_attn, dtype=np.float32)
    b_attn = np.asarray(b_attn, dtype=np.float32)
    w_proj = np.asarray(w_proj, dtype=np.float32)

    xT = np.ascontiguousarray(x.reshape(BT, C).T)          # [C, BT]
    # xp8[tb, p, k2, s2, c] = xT[k2*256 + s2*128 + p, tb*512 + c]
    xp8 = np.ascontiguousarray(
        xT.reshape(KT2, 2, P, NTB, 512).transpose(3, 2, 0, 1, 4)).astype(E4NP)
    # xpb[tb, p, kt, c] = xT[kt*128 + p, tb*512 + c]
    xpb = np.ascontiguousarray(
        xT.reshape(KT, P, NTB, 512).transpose(2, 1, 0, 3)).astype(BFNP)

    wpb = np.ascontiguousarray(
        w_proj.reshape(KT, P, C).transpose(1, 0, 2)).astype(BFNP)

    in_maps = []
    for c in range(NCORES):
        cols = slice(P * c, P * (c + 1))

        def wslice8(off):
            w = WS * w_attn[:, off + P * c: off + P * (c + 1)]  # [1024, 128]
            return np.ascontiguousarray(
                w.reshape(KT2, 2, P, P).transpose(2, 0, 1, 3)).astype(E4NP)

        wv = w_attn[:, 2 * C + P * c: 2 * C + P * (c + 1)]
        wvb = np.ascontiguousarray(
            wv.reshape(KT, P, P).transpose(1, 0, 2)).astype(BFNP)

        in_maps.append({
            "xp8": xp8,
            "xpb": xpb,
            "wq8": wslice8(0),
            "wk8": wslice8(C),
            "wvb": wvb,
            "wpb": wpb,
            "wprb": np.ascontiguousarray(w_proj[cols, :]).astype(BFNP),
            "bq": (WS * np.ascontiguousarray(b_attn[cols])).reshape(P, 1),
            "bk": (WS * np.ascontiguousarray(
                b_attn[C + P * c: C + P * (c + 1)])).reshape(P, 1),
        })
    return in_maps


def kernel(x, w_attn, b_attn, w_proj, b_proj):
    from concourse.bass_utils import run_bass_kernel_spmd

    if "nc" not in _CACHED:
        _CACHED["nc"] = _build_nc()
    nc = _CACHED["nc"]

    in_maps = _prep_inputs(x, w_attn, b_attn, w_proj)
    res = run_bass_kernel_spmd(nc, in_maps, core_ids=list(range(NCORES)))

    # batches 0-2: core c holds tokens [h*1024 + c*128, +128) of each
    # half h; batch 3 comes back as row-parallel partials (bf16)
    y = np.empty((B, T, C), dtype=np.float32)
    for c in range(NCORES):
        part = res.results[c]["yp"]          # [3, 2, 128, C] f32
        for h in range(2):
            y[:3, h * (T // 2) + c * 128: h * (T // 2) + (c + 1) * 128, :] = \
                part[:, h]
    acc = res.results[0]["ypl"].astype(np.float32)
    for c in range(1, NCORES):
        acc = acc + res.results[c]["ypl"].astype(np.float32)
    y[B - 1] = acc
    # b_v folds into a constant output shift (softmax weights sum to 1)
    bias = np.asarray(b_proj, dtype=np.float32) + \
        np.asarray(b_attn, dtype=np.float32)[2 * C:] @ np.asarray(
            w_proj, dtype=np.float32)
    y += bias
    return y
